# revision 2
# baseline (speedup 1.0000x reference)
"""Trainium2 Bass kernel for nn_DGLGATNE (GNN message passing, 8 NeuronCores).

Strategy (dst-sharded): core c owns dst nodes [1024c, 1024(c+1)) for all 4 edge
types.  Per core, dsts are processed in H=2 pipelined halves of 512:
  1. Per-edge message gather from an fp16 copy of node_type_embeddings
     ([500000, 128] fp16; 64-B per-edge payload) via the custom dma_gather
     SWDGE ucode, chunked 16 ways for the int16 index limit and spread over
     4 SWDGE queues.  Edges sorted by (half, chunk, type, dst) on the host.
     SWDGE desc-gen on the Pool engine (~2.5 ns/desc) is the kernel's
     critical resource; everything else is scheduled to hide under it.
  2. Segment-sum into per-half aggT [128 (t,u), 512 dst] f32 in PSUM via
     per-tile matmuls: out = msgs^T @ onehot, onehot built on-chip with
     is_equal(iota, dst_rel); dst windows follow a host-computed static
     schedule shared by all 8 cores (SPMD).
  3. When half h's edge stream ends, its attention + per-type transform +
     L2-normalize + store run on PE/DVE/Act while half h+1's gathers continue
     on the Pool engine (tail hiding).
  4. node_embeddings[output_nodes] gathered f32 in two stages per half
     (chunk-sorted into a DRAM scratch, then one in-order gather), with the
     stage-1 calls interleaved into the edge-gather stream.

Host-side preprocessing is integer-only index manipulation (sharding /
sorting / padding) plus a dtype cast of the embedding table; no float
gather/compute happens on the host.
"""
import sys
import contextlib

for _p in ('/opt/trn_rl_repo', '/root/.axon_site/_ro/trn_rl_repo'):
    if _p not in sys.path:
        sys.path.insert(0, _p)

import numpy as np

import concourse.bass as bass
import concourse.mybir as mybir
import concourse.tile as tile
import concourse.bacc as bacc
import concourse.ap_utils as ap_utils
from concourse._compat import exact_div, round_up_to_multiple
from concourse.bass_utils import run_bass_kernel_spmd

# problem shapes (hardcoded per spec)
NUM_NODES = 500000
D = 128
U = 32
T = 4
A = 32
N_SRC = 65536
B = 8192
E = 262144
NCORES = 8
BC = B // NCORES          # 1024 dst per core
H = 2                     # pipelined dst halves per core
BH = BC // H              # 512 dst per half
CHUNK = 32768             # int16 index reach per dma_gather call
NCH = (NUM_NODES + CHUNK - 1) // CHUNK   # 16
WIN = 64                  # dst window stride (windows are 128 wide)
OH_BATCH = 16             # tiles per onehot is_equal op
F16 = mybir.dt.float16
F32 = mybir.dt.float32
I16 = mybir.dt.int16


def _wrap16(idx, nrep=8):
    """[n] -> [16*nrep, n/16] wrapped (i -> [i%16, i//16]), replicated."""
    n = len(idx)
    assert n % 16 == 0
    w = idx.reshape(n // 16, 16).T.astype(np.int16)
    return np.tile(w, (nrep, 1))


def _dma_gather_raw(gp, out_ap, in_ap, idxs_ap, num_idxs, elem_size, elem_step,
                    queue_num=0):
    """bass dma_gather minus the elem%256 transpose-only restriction."""
    assert idxs_ap.dtype == I16
    assert in_ap.dtype == out_ap.dtype
    assert in_ap.space == bass.MemorySpace.DRAM
    assert ap_utils.ap_is_contiguous(in_ap.ap[1:])
    assert ap_utils.ap_is_contiguous(out_ap.ap[1:])
    assert ap_utils.ap_is_contiguous(idxs_ap.ap[1:])
    assert in_ap.ap[-1][1] == out_ap.ap[-1][1] == elem_size
    assert out_ap.ap[0][1] * out_ap.ap[1][1] == round_up_to_multiple(num_idxs, 128)
    assert in_ap.ap[0][0] == elem_step
    stride_bytes = elem_step * mybir.dt.size(in_ap.dtype)
    stride_bytes_256 = exact_div(stride_bytes, 256)
    assert stride_bytes_256 < 256
    _in_ap = gp.lower_ap_dma(in_ap, for_custom_bir_dma=True)
    _idxs_ap = gp.lower_ap(idxs_ap)
    _out_ap = gp.lower_ap(out_ap)
    return gp.add_instruction(
        mybir.InstDMAGatherAnt(
            name=gp.bass.get_next_instruction_name(),
            ins=[*_in_ap, _idxs_ap, gp.lower_val_access(gp.to_reg(num_idxs))],
            outs=[_out_ap],
            transpose=False,
            num_idxs=num_idxs,
            elem_size=elem_size,
            stride_bytes_256=stride_bytes_256,
            gen_mode=0,
            single_packet=False,
            queue_num=queue_num,
            sbuf_tokens_per_rank=0,
            sbuf_free_dim_per_rank=0,
            sbuf_free_dim_pad_per_rank=0,
            sbuf_byte_offset=0,
        )
    )


def _bc_ap(t, ap_dims):
    """Build an AP on tile t's tensor with explicit [step, count] dims."""
    return bass.AP(t.tensor, t.offset, ap_dims)


def _prep(inputs):
    """Integer-only host preprocessing -> per-core upload arrays + shared
    static schedule."""
    input_nodes = np.asarray(inputs["input_nodes"]).astype(np.int64)
    output_nodes = np.asarray(inputs["output_nodes"]).astype(np.int64)
    edge_src = np.asarray(inputs["edge_src"]).astype(np.int64)
    edge_dst = np.asarray(inputs["edge_dst"]).astype(np.int64)

    gidx = input_nodes[edge_src]              # [T, E] table row per edge
    chunk_of = gidx // CHUNK
    core_of = edge_dst // BC

    # group edges by (half, chunk, type) per core, dst-sorted
    glists = {}
    for t in range(T):
        dst_loc = edge_dst[t] - core_of[t] * BC
        half_of = dst_loc // BH
        key = (core_of[t] * H + half_of) * NCH + chunk_of[t]
        order = np.lexsort((edge_dst[t], key))
        ks = key[order]
        bounds = np.searchsorted(ks, np.arange(NCORES * H * NCH + 1))
        for c in range(NCORES):
            for h in range(H):
                for ch in range(NCH):
                    s, e = (bounds[(c * H + h) * NCH + ch],
                            bounds[(c * H + h) * NCH + ch + 1])
                    sel = order[s:e]
                    glists[(h, ch, t, c)] = (
                        gidx[t][sel] % CHUNK,
                        dst_loc[sel] - BH * h,
                    )

    # joint window schedule per (h, ch, t) across all 8 cores
    group_order = [(h, ch, t)
                   for h in range(H) for ch in range(NCH) for t in range(T)]
    sched = {}          # (h,ch,t) -> list of window ids
    core_idx = [[] for _ in range(NCORES)]   # gather idx slot streams
    core_rel = [[] for _ in range(NCORES)]   # dst_rel slot streams
    for (h, ch, t) in group_order:
        lists = [glists[(h, ch, t, c)] for c in range(NCORES)]
        n = [len(x[1]) for x in lists]
        ptr = [0] * NCORES
        wins = []
        while any(ptr[c] < n[c] for c in range(NCORES)):
            w = min(
                int(lists[c][1][ptr[c]]) // WIN
                for c in range(NCORES) if ptr[c] < n[c]
            )
            w = min(w, (BH - 128) // WIN)
            hi = WIN * w + 128
            for c in range(NCORES):
                li, dl = lists[c]
                j0 = ptr[c]
                j1 = min(int(np.searchsorted(dl, hi)), j0 + 128, n[c])
                take = j1 - j0
                idx_slot = np.zeros(128, np.int64)
                rel_slot = np.full(128, -1.0, np.float32)
                idx_slot[:take] = li[j0:j1]
                rel_slot[:take] = dl[j0:j1] - WIN * w
                core_idx[c].append(idx_slot)
                core_rel[c].append(rel_slot)
                ptr[c] = j1
            wins.append(w)
        sched[(h, ch, t)] = wins

    total_tiles = sum(len(sched[g]) for g in group_order)

    eidx = []
    rel = []
    for c in range(NCORES):
        slots = np.concatenate(core_idx[c]) if core_idx[c] else np.zeros(0, np.int64)
        eidx.append(_wrap16(slots))
        rel.append(np.stack(core_rel[c], axis=1).astype(np.float16))  # [128, ntiles]

    # output-node (emb) gather per half: stage 1 chunk-sorted into a DRAM
    # scratch, stage 2 in-order
    on_ch = {}
    for c in range(NCORES):
        for h in range(H):
            on_ch[(c, h)] = output_nodes[BC * c + BH * h:
                                         BC * c + BH * (h + 1)] // CHUNK
    M = {}      # (h, ch) -> padded stage-1 count (shared across cores)
    for h in range(H):
        for ch in range(NCH):
            mx = max(int(np.sum(on_ch[(c, h)] == ch)) for c in range(NCORES))
            M[(h, ch)] = max(round_up_to_multiple(mx, 16), 16)
    kch = {k: int(-(-m // 128)) for k, m in M.items()}   # scratch blocks
    scr_base = {}
    acc = 0
    for h in range(H):
        for ch in range(NCH):
            scr_base[(h, ch)] = acc
            acc += 128 * kch[(h, ch)]
    scr_rows = acc
    assert scr_rows <= CHUNK

    gidx1 = []   # stage-1 idx arrays, concatenated over (h, ch)
    gmap = []    # stage-2 idx (scratch slot per local b), [BC] (h-major)
    for c in range(NCORES):
        parts = []
        slots_all = []
        for h in range(H):
            nodes = output_nodes[BC * c + BH * h:BC * c + BH * (h + 1)]
            chs = on_ch[(c, h)]
            order = np.argsort(chs, kind="stable")
            slots = np.zeros(BH, np.int64)
            for ch in range(NCH):
                sel = order[np.searchsorted(chs[order], ch):
                            np.searchsorted(chs[order], ch + 1)]
                col = np.zeros(M[(h, ch)], np.int64)
                col[:len(sel)] = nodes[sel] % CHUNK
                parts.append(col)
                slots[sel] = scr_base[(h, ch)] + np.arange(len(sel))
            slots_all.append(slots)
        gidx1.append(_wrap16(np.concatenate(parts)))
        gmap.append(_wrap16(np.concatenate(slots_all)))

    shared = dict(
        group_order=group_order,
        sched=sched,
        total_tiles=total_tiles,
        M=M,
        kch=kch,
        scr_base=scr_base,
        scr_rows=scr_rows,
    )
    percore = [
        dict(eidx=eidx[c], rel=rel[c], gidx1=gidx1[c], gmap=gmap[c])
        for c in range(NCORES)
    ]
    return shared, percore


def _build(nc, S):
    """Build the bass program (identical for all cores; data differs)."""
    group_order = S["group_order"]
    sched = S["sched"]
    TOT_TILES = S["total_tiles"]
    TOT_SLOTS = TOT_TILES * 128
    M, KCH, SCR_BASE, SCR_ROWS = S["M"], S["kch"], S["scr_base"], S["scr_rows"]
    M1_TOT = sum(M.values())

    table = nc.dram_tensor("table", [NUM_NODES, T * U], F16, kind="ExternalInput")
    embt = nc.dram_tensor("embt", [NUM_NODES, D], F32, kind="ExternalInput")
    eidx_t = nc.dram_tensor("eidx", [128, TOT_SLOTS // 16], I16, kind="ExternalInput")
    rel_t = nc.dram_tensor("rel", [128, TOT_TILES], F16, kind="ExternalInput")
    nrel_t = nc.dram_tensor("nrel", [128, TOT_TILES], F16, kind="ExternalInput")
    gidx1_t = nc.dram_tensor("gidx1", [128, M1_TOT // 16], I16, kind="ExternalInput")
    gmap_t = nc.dram_tensor("gmap", [128, BC // 16], I16, kind="ExternalInput")
    iota_t = nc.dram_tensor("iota", [128, 128], F16, kind="ExternalInput")
    w1s_t = nc.dram_tensor("w1s", [128, A], F32, kind="ExternalInput")
    w2s_t = nc.dram_tensor("w2s", [128, 1], F32, kind="ExternalInput")
    ind4_t = nc.dram_tensor("ind4", [128, T], F32, kind="ExternalInput")
    ind4t_t = nc.dram_tensor("ind4t", [T, 128], F32, kind="ExternalInput")
    ones4_t = nc.dram_tensor("ones4", [T, 1], F32, kind="ExternalInput")
    ones1_t = nc.dram_tensor("ones1", [1, 128], F32, kind="ExternalInput")
    isum_t = nc.dram_tensor("isum", [128, U], F32, kind="ExternalInput")
    wsb_t = nc.dram_tensor("wsb", [U, T * D], F32, kind="ExternalInput")
    out_t = nc.dram_tensor("out", [BC, T, D], F32, kind="ExternalOutput")
    scratch = nc.dram_tensor("embscr", [SCR_ROWS, D], F32, kind="Internal")

    # per-group eidx column starts (in idx slots / 16)
    g_ntk = [len(sched[g]) for g in group_order]
    g_col = np.concatenate([[0], np.cumsum([n * 128 for n in g_ntk])])
    g_tcol = np.concatenate([[0], np.cumsum(g_ntk)])
    NGRP_H = len(group_order) // H

    with tile.TileContext(nc) as tc:
      with contextlib.ExitStack() as ctx2:
        cpool = ctx2.enter_context(tc.tile_pool(name="const", bufs=1))
        # edge idx upload split per group so the first gather starts at once
        eidx_sb = cpool.tile([128, TOT_SLOTS // 16], I16)
        for gi in range(len(group_order)):
            c0, c1 = g_col[gi] // 16, g_col[gi + 1] // 16
            if c1 > c0:
                nc.sync.dma_start(eidx_sb[:, c0:c1], eidx_t[:, c0:c1])
        rel_sb = cpool.tile([128, TOT_TILES], F16)
        nrel_sb = cpool.tile([128, TOT_TILES], F16)
        hcut = int(g_tcol[NGRP_H])
        nc.sync.dma_start(rel_sb[:, 0:hcut], rel_t[:, 0:hcut])
        nc.sync.dma_start(nrel_sb[:, 0:hcut], nrel_t[:, 0:hcut])
        nc.sync.dma_start(rel_sb[:, hcut:], rel_t[:, hcut:])
        nc.sync.dma_start(nrel_sb[:, hcut:], nrel_t[:, hcut:])
        gidx1_sb = cpool.tile([128, M1_TOT // 16], I16)
        nc.sync.dma_start(gidx1_sb[:], gidx1_t[:])
        gmap_sb = cpool.tile([128, BC // 16], I16)
        nc.sync.dma_start(gmap_sb[:], gmap_t[:])
        iota_sb = cpool.tile([128, 128], F16)
        nc.sync.dma_start(iota_sb[:], iota_t[:])
        w1s_sb = cpool.tile([128, A], F32)
        nc.sync.dma_start(w1s_sb[:], w1s_t[:])
        w2s_sb = cpool.tile([128, 1], F32)
        nc.sync.dma_start(w2s_sb[:], w2s_t[:])
        ind4_sb = cpool.tile([128, T], F32)
        nc.sync.dma_start(ind4_sb[:], ind4_t[:])
        ind4t_sb = cpool.tile([T, 128], F32)
        nc.sync.dma_start(ind4t_sb[:], ind4t_t[:])
        ones4_sb = cpool.tile([T, 1], F32)
        nc.sync.dma_start(ones4_sb[:], ones4_t[:])
        ones1_sb = cpool.tile([1, 128], F32)
        nc.sync.dma_start(ones1_sb[:], ones1_t[:])
        isum_sb = cpool.tile([128, U], F32)
        nc.sync.dma_start(isum_sb[:], isum_t[:])
        wsb_sb = cpool.tile([U, T * D], F32)
        nc.sync.dma_start(wsb_sb[:], wsb_t[:])
        aggT_sb = [cpool.tile([128, BH], F32, name=f"aggsb{h}") for h in range(H)]
        embB = [cpool.tile([128, BH // 128, D], F32, name=f"embB{h}")
                for h in range(H)]

        # ---- emb gather stage 1 (interleaved into the edge stream) -------
        scr_writes = [[] for _ in range(H)]
        g1pool = ctx2.enter_context(tc.tile_pool(name="g1", bufs=3))
        _off1 = [0]

        def _emb_stage1_call(h, ch):
            m, k = M[(h, ch)], KCH[(h, ch)]
            off1 = _off1[0]
            ge = g1pool.tile([128, k, D], F32, name=f"ge{h}_{ch}", tag="ge")
            in_view = embt[ch * CHUNK:min(NUM_NODES, (ch + 1) * CHUNK), :]
            _dma_gather_raw(nc.gpsimd, ge[:], in_view,
                            gidx1_sb[:, off1 // 16:(off1 + m) // 16],
                            m, D, D, queue_num=ch % 4)
            wr = nc.sync.dma_start(
                scratch[SCR_BASE[(h, ch)]:SCR_BASE[(h, ch)] + 128 * k, :]
                .rearrange("(k p) d -> p k d", p=128),
                ge[:],
            )
            scr_writes[h].append(wr)
            _off1[0] = off1 + m

        # pools for the edge stream + attention/output (kept open throughout)
        mpool = ctx2.enter_context(tc.tile_pool(name="msgs", bufs=10))
        ohpool = ctx2.enter_context(tc.tile_pool(name="ohp", bufs=6))
        aggps = ctx2.enter_context(tc.psum_pool(name="aggps", bufs=2))
        php = ctx2.enter_context(tc.psum_pool(name="ph", bufs=2))
        wkp = ctx2.enter_context(tc.tile_pool(name="wk", bufs=2))
        pop = ctx2.enter_context(tc.psum_pool(name="po", bufs=2))
        owp = ctx2.enter_context(tc.tile_pool(name="ow", bufs=2))

        aggT_ps = [aggps.tile([128, BH], F32, name=f"aggps{h}", tag=f"agg{h}")
                   for h in range(H)]
        for h in range(H):
            nc.vector.memset(aggT_ps[h][:], 0.0)
        # pre-clear msgs slots once (stale tails must stay finite)
        max_ntk = max(g_ntk) if g_ntk else 1
        for i in range(10):
            mz = mpool.tile([128, max_ntk, U], F16, name=f"mz{i}", tag="msgs")
            nc.vector.memset(mz.rearrange("p a b -> p (a b)"), 0.0)

        def _attention_output(h):
            """Attention over types + transform + emb add + normalize +
            store for half h (aggT_ps[h] complete at this point)."""
            nc.vector.tensor_copy(aggT_sb[h][:], aggT_ps[h][:])
            psumH = php.tile([128, BH], F32, tag="ph")
            for t in range(T):
                nc.tensor.matmul(
                    psumH[32 * t:32 * (t + 1), :],
                    w1s_sb[32 * t:32 * (t + 1), :],
                    aggT_sb[h][32 * t:32 * (t + 1), :],
                    start=True, stop=True, tile_position=(32 * t, 32 * t))
            H_sb = wkp.tile([128, BH], F32, tag="wk")
            nc.scalar.activation(H_sb[:], psumH[:],
                                 mybir.ActivationFunctionType.Tanh)
            HW2 = wkp.tile([128, BH], F32, tag="wk")
            nc.vector.tensor_scalar(HW2[:], H_sb[:], w2s_sb[:, 0:1], None,
                                    mybir.AluOpType.mult)
            psumS = php.tile([128, BH], F32, tag="ph")
            nc.tensor.matmul(psumS[0:T, :], ind4_sb[:], HW2[:],
                             start=True, stop=True)
            E4 = wkp.tile([T, BH], F32, tag=f"e4_{h}", bufs=1)
            nc.scalar.activation(E4[:], psumS[0:T, :],
                                 mybir.ActivationFunctionType.Exp)
            psumD = php.tile([128, BH], F32, tag="ph")
            nc.tensor.matmul(psumD[0:1, :], ones4_sb[:], E4[:],
                             start=True, stop=True)
            r_sb = wkp.tile([1, BH], F32, tag=f"r_{h}", bufs=1)
            nc.vector.reciprocal(r_sb[:], psumD[0:1, :])
            psumE = php.tile([128, BH], F32, tag="ph")
            nc.tensor.matmul(psumE[:], ind4t_sb[:], E4[:],
                             start=True, stop=True)
            psumR = php.tile([128, BH], F32, tag="ph")
            nc.tensor.matmul(psumR[:], ones1_sb[:], r_sb[:],
                             start=True, stop=True)
            M_sb = wkp.tile([128, BH], F32, tag="wk")
            nc.vector.tensor_tensor(M_sb[:], aggT_sb[h][:], psumE[:],
                                    mybir.AluOpType.mult)
            nc.vector.tensor_tensor(M_sb[:], M_sb[:], psumR[:],
                                    mybir.AluOpType.mult)
            psumC = php.tile([128, BH], F32, tag="ph")
            nc.tensor.matmul(psumC[0:U, :], isum_sb[:], M_sb[:],
                             start=True, stop=True)
            combT = wkp.tile([U, BH], F32, tag=f"comb_{h}", bufs=1)
            nc.vector.tensor_copy(combT[:], psumC[0:U, :])

            for j in range(BH // 128):
                psumO = pop.tile([128, T * D], F32, tag="po")
                for t in range(T):
                    nc.tensor.matmul(
                        psumO[:, D * t:D * (t + 1)],
                        combT[:, 128 * j:128 * (j + 1)],
                        wsb_sb[:, D * t:D * (t + 1)],
                        start=True, stop=True)
                O_sb = owp.tile([128, T * D], F32, tag="osb")
                eslice = embB[h][:, j, :]
                emb_bc = _bc_ap(eslice,
                                [eslice.ap[0], [0, T], eslice.ap[1]])
                nc.vector.tensor_tensor(
                    O_sb.rearrange("p (t d) -> p t d", t=T),
                    psumO.rearrange("p (t d) -> p t d", t=T), emb_bc,
                    mybir.AluOpType.add)
                sq = owp.tile([128, T * D], F32, tag="sq")
                nc.scalar.activation(sq[:], O_sb[:],
                                     mybir.ActivationFunctionType.Square)
                ss = owp.tile([128, T], F32, tag="ss")
                nc.vector.tensor_reduce(
                    ss[:], sq.rearrange("p (t d) -> p t d", t=T),
                    mybir.AxisListType.X, mybir.AluOpType.add)
                nrm = owp.tile([128, T], F32, tag="nrm")
                nc.scalar.activation(nrm[:], ss[:],
                                     mybir.ActivationFunctionType.Sqrt)
                nc.vector.tensor_scalar(nrm[:], nrm[:], 1e-12, None,
                                        mybir.AluOpType.max)
                rn = owp.tile([128, T], F32, tag="rn")
                nc.vector.reciprocal(rn[:], nrm[:])
                rn_bc = _bc_ap(rn, [rn.ap[0], rn.ap[1], [0, D]])
                nc.vector.tensor_tensor(
                    O_sb.rearrange("p (t d) -> p t d", t=T),
                    O_sb.rearrange("p (t d) -> p t d", t=T), rn_bc,
                    mybir.AluOpType.mult)
                nc.sync.dma_start(
                    out_t.rearrange("(j p) t d -> p j (t d)", p=128)
                    [:, (BH // 128) * h + j, :],
                    O_sb[:])

        # ---- edge-message gather + matmul scatter, h-major ---------------
        ohcnt = 0
        for gi, (h, ch, t) in enumerate(group_order):
            ntk = g_ntk[gi]
            if ntk > 0:
                col = int(g_col[gi])
                tcol = int(g_tcol[gi])
                msgs = mpool.tile([128, max_ntk, U], F16, name=f"m{gi}",
                                  tag="msgs")
                in_view = table[ch * CHUNK:min(NUM_NODES, (ch + 1) * CHUNK),
                                U * t:U * (t + 1)]
                _dma_gather_raw(
                    nc.gpsimd,
                    msgs[:, :ntk, :],
                    in_view,
                    eidx_sb[:, col // 16:(col + ntk * 128) // 16],
                    ntk * 128, U, T * U, queue_num=gi % 4)
                for b0 in range(0, ntk, OH_BATCH):
                    nb = min(OH_BATCH, ntk - b0)
                    oh = ohpool.tile([128, OH_BATCH, 128], F16,
                                     name=f"oh{gi}_{b0}", tag="oh")
                    iota_bc = _bc_ap(iota_sb,
                                     [iota_sb.ap[0], [0, nb], [1, 128]])
                    if ohcnt % 3 != 2:
                        rsl = rel_sb[:, tcol + b0:tcol + b0 + nb]
                        rel_bc = _bc_ap(
                            rsl, [rsl.ap[0], [rsl.ap[1][0], nb], [0, 128]])
                        nc.vector.tensor_tensor(
                            oh[:, :nb, :], iota_bc, rel_bc,
                            mybir.AluOpType.is_equal)
                    else:
                        # onehot = Relu(1 - |iota - rel|) on the Scalar engine
                        for k2 in range(nb):
                            nc.scalar.activation(
                                oh[:, k2, :], iota_sb[:],
                                mybir.ActivationFunctionType.Abs,
                                bias=nrel_sb[:, tcol + b0 + k2:
                                             tcol + b0 + k2 + 1])
                        nc.scalar.activation(
                            oh[:, :nb, :], oh[:, :nb, :],
                            mybir.ActivationFunctionType.Relu,
                            bias=1.0, scale=-1.0)
                    ohcnt += 1
                    for k in range(nb):
                        w = sched[(h, ch, t)][b0 + k]
                        assert WIN * w + 128 <= BH
                        nc.tensor.matmul(
                            aggT_ps[h][32 * t:32 * (t + 1),
                                       WIN * w:WIN * w + 128],
                            msgs[:, b0 + k, :], oh[:, k, :],
                            start=False, stop=False, skip_group_check=True,
                            tile_position=(0, 32 * t))
            # interleave emb stage-1 gathers: after the last type of chunk ch
            if t == T - 1:
                _emb_stage1_call(h, ch)
            # end of half h: emb stage 2, then attention + output for h
            if (gi + 1) % NGRP_H == 0:
                g2 = _dma_gather_raw(
                    nc.gpsimd, embB[h][:], scratch[:, :],
                    gmap_sb[:, (BH // 16) * h:(BH // 16) * (h + 1)],
                    BH, D, D, queue_num=0)
                for wr in scr_writes[h]:
                    bass._add_dep_helper(
                        g2.ins, wr.ins, sync=True,
                        reason="emb stage2 waits on scratch writes")
                _attention_output(h)
    nc.finalize()
    return nc


def kernel(**inputs):
    shared, percore = _prep(inputs)

    table_f16 = np.ascontiguousarray(
        np.asarray(inputs["node_type_embeddings"], dtype=np.float32)
        .reshape(NUM_NODES, T * U)).astype(np.float16)
    embt = np.ascontiguousarray(
        np.asarray(inputs["node_embeddings"], dtype=np.float32))
    trans_w = np.asarray(inputs["trans_w"], dtype=np.float32)
    trans_w_s1 = np.asarray(inputs["trans_w_s1"], dtype=np.float32)
    trans_w_s2 = np.asarray(inputs["trans_w_s2"], dtype=np.float32)

    w1s = np.ascontiguousarray(trans_w_s1.reshape(T * U, A))
    w2s = np.ascontiguousarray(trans_w_s2.reshape(T * A, 1))
    ind4 = np.zeros((128, T), np.float32)
    for t in range(T):
        ind4[32 * t:32 * (t + 1), t] = 1.0
    ind4t = np.ascontiguousarray(ind4.T)
    ones4 = np.ones((T, 1), np.float32)
    ones1 = np.ones((1, 128), np.float32)
    isum = np.zeros((128, U), np.float32)
    for t in range(T):
        isum[32 * t:32 * (t + 1), :] = np.eye(U, dtype=np.float32)
    wsb = np.ascontiguousarray(
        trans_w.transpose(1, 0, 2).reshape(U, T * D))
    iota = np.tile(np.arange(128, dtype=np.float16), (128, 1))

    nc = bacc.Bacc(trn_type="TRN2", num_swdge_queues=4,
                   dynamic_dma_scratch_size=32768)
    _build(nc, shared)

    common = dict(table=table_f16, embt=embt, iota=iota, w1s=w1s, w2s=w2s,
                  ind4=ind4, ind4t=ind4t, ones4=ones4, ones1=ones1,
                  isum=isum, wsb=wsb)
    in_maps = []
    for c in range(NCORES):
        pc = percore[c]
        in_maps.append(dict(common, eidx=pc["eidx"], rel=pc["rel"],
                            nrel=-pc["rel"],
                            gidx1=pc["gidx1"], gmap=pc["gmap"]))

    res = run_bass_kernel_spmd(nc, in_maps, core_ids=list(range(NCORES)))
    out = np.concatenate([r["out"] for r in res.results], axis=0)
    return out.astype(np.float32)


if __name__ == "__main__":
    import reference
    inputs = reference.setup_inputs()
    inputs = {k: np.asarray(v) for k, v in inputs.items()}
    out = kernel(**inputs)
    print("out", out.shape, out.dtype)


# revision 3
# speedup vs baseline: 1.2146x; 1.2146x over previous
"""Trainium2 Bass kernel for nn_DGLGATNE (GNN message passing, 8 NeuronCores).

Strategy (dst-sharded): core c owns dst nodes [1024c, 1024(c+1)) for all 4 edge
types.  Per core, dsts are processed in H=2 pipelined halves of 512:
  1. Per-edge message gather from an fp16 copy of node_type_embeddings
     ([500000, 128] fp16; 64-B per-edge payload) via the custom dma_gather
     SWDGE ucode, chunked 16 ways for the int16 index limit and spread over
     4 SWDGE queues.  Edges sorted by (half, chunk, type, dst) on the host.
     SWDGE desc-gen on the Pool engine (~2.5 ns/desc) is the kernel's
     critical resource; everything else is scheduled to hide under it.
  2. Segment-sum into per-half aggT [128 (t,u), 512 dst] f32 in PSUM via
     per-tile matmuls: out = msgs^T @ onehot, onehot built on-chip with
     is_equal(iota, dst_rel); dst windows follow a host-computed static
     schedule shared by all 8 cores (SPMD).
  3. When half h's edge stream ends, its attention + per-type transform +
     L2-normalize + store run on PE/DVE/Act while half h+1's gathers continue
     on the Pool engine (tail hiding).
  4. node_embeddings[output_nodes] gathered f32 in two stages per half
     (chunk-sorted into a DRAM scratch, then one in-order gather), with the
     stage-1 calls interleaved into the edge-gather stream.

Host-side preprocessing is integer-only index manipulation (sharding /
sorting / padding) plus a dtype cast of the embedding table; no float
gather/compute happens on the host.
"""
import sys
import contextlib

for _p in ('/opt/trn_rl_repo', '/root/.axon_site/_ro/trn_rl_repo'):
    if _p not in sys.path:
        sys.path.insert(0, _p)

import numpy as np

import concourse.bass as bass
import concourse.mybir as mybir
import concourse.tile as tile
import concourse.bacc as bacc
import concourse.ap_utils as ap_utils
from concourse._compat import exact_div, round_up_to_multiple
from concourse.bass_utils import run_bass_kernel_spmd

# problem shapes (hardcoded per spec)
NUM_NODES = 500000
D = 128
U = 32
T = 4
A = 32
N_SRC = 65536
B = 8192
E = 262144
NCORES = 8
BC = B // NCORES          # 1024 dst per core
H = 2                     # pipelined dst halves per core
BH = BC // H              # 512 dst per half
CHUNK = 32768             # int16 index reach per dma_gather call
NCH = (NUM_NODES + CHUNK - 1) // CHUNK   # 16
WIN = 64                  # dst window stride (windows are 128 wide)
OH_BATCH = 16             # tiles per onehot is_equal op
F16 = mybir.dt.float16
F32 = mybir.dt.float32
I16 = mybir.dt.int16


def _wrap16(idx, nrep=8):
    """[n] -> [16*nrep, n/16] wrapped (i -> [i%16, i//16]), replicated."""
    n = len(idx)
    assert n % 16 == 0
    w = idx.reshape(n // 16, 16).T.astype(np.int16)
    return np.tile(w, (nrep, 1))


def _dma_gather_raw(gp, out_ap, in_ap, idxs_ap, num_idxs, elem_size, elem_step,
                    queue_num=0):
    """bass dma_gather minus the elem%256 transpose-only restriction."""
    assert idxs_ap.dtype == I16
    assert in_ap.dtype == out_ap.dtype
    assert in_ap.space == bass.MemorySpace.DRAM
    assert ap_utils.ap_is_contiguous(in_ap.ap[1:])
    assert ap_utils.ap_is_contiguous(out_ap.ap[1:])
    assert ap_utils.ap_is_contiguous(idxs_ap.ap[1:])
    assert in_ap.ap[-1][1] == out_ap.ap[-1][1] == elem_size
    assert out_ap.ap[0][1] * out_ap.ap[1][1] == round_up_to_multiple(num_idxs, 128)
    assert in_ap.ap[0][0] == elem_step
    stride_bytes = elem_step * mybir.dt.size(in_ap.dtype)
    stride_bytes_256 = exact_div(stride_bytes, 256)
    assert stride_bytes_256 < 256
    _in_ap = gp.lower_ap_dma(in_ap, for_custom_bir_dma=True)
    _idxs_ap = gp.lower_ap(idxs_ap)
    _out_ap = gp.lower_ap(out_ap)
    return gp.add_instruction(
        mybir.InstDMAGatherAnt(
            name=gp.bass.get_next_instruction_name(),
            ins=[*_in_ap, _idxs_ap, gp.lower_val_access(gp.to_reg(num_idxs))],
            outs=[_out_ap],
            transpose=False,
            num_idxs=num_idxs,
            elem_size=elem_size,
            stride_bytes_256=stride_bytes_256,
            gen_mode=0,
            single_packet=False,
            queue_num=queue_num,
            sbuf_tokens_per_rank=0,
            sbuf_free_dim_per_rank=0,
            sbuf_free_dim_pad_per_rank=0,
            sbuf_byte_offset=0,
        )
    )


def _bc_ap(t, ap_dims):
    """Build an AP on tile t's tensor with explicit [step, count] dims."""
    return bass.AP(t.tensor, t.offset, ap_dims)


def _prep(inputs):
    """Integer-only host preprocessing -> per-core upload arrays + shared
    static schedule."""
    input_nodes = np.asarray(inputs["input_nodes"]).astype(np.int64)
    output_nodes = np.asarray(inputs["output_nodes"]).astype(np.int64)
    edge_src = np.asarray(inputs["edge_src"]).astype(np.int64)
    edge_dst = np.asarray(inputs["edge_dst"]).astype(np.int64)

    gidx = input_nodes[edge_src]              # [T, E] table row per edge
    chunk_of = gidx // CHUNK
    core_of = edge_dst // BC

    # group edges by (half, chunk, type) per core, dst-sorted
    glists = {}
    for t in range(T):
        dst_loc = edge_dst[t] - core_of[t] * BC
        half_of = dst_loc // BH
        key = (core_of[t] * H + half_of) * NCH + chunk_of[t]
        order = np.lexsort((edge_dst[t], key))
        ks = key[order]
        bounds = np.searchsorted(ks, np.arange(NCORES * H * NCH + 1))
        for c in range(NCORES):
            for h in range(H):
                for ch in range(NCH):
                    s, e = (bounds[(c * H + h) * NCH + ch],
                            bounds[(c * H + h) * NCH + ch + 1])
                    sel = order[s:e]
                    glists[(h, ch, t, c)] = (
                        gidx[t][sel] % CHUNK,
                        dst_loc[sel] - BH * h,
                    )

    # joint window schedule per (h, ch, t) across all 8 cores
    group_order = [(h, ch, t)
                   for h in range(H) for ch in range(NCH) for t in range(T)]
    sched = {}          # (h,ch,t) -> list of window ids
    core_idx = [[] for _ in range(NCORES)]   # gather idx slot streams
    core_rel = [[] for _ in range(NCORES)]   # dst_rel slot streams
    for (h, ch, t) in group_order:
        lists = [glists[(h, ch, t, c)] for c in range(NCORES)]
        n = [len(x[1]) for x in lists]
        ptr = [0] * NCORES
        wins = []
        while any(ptr[c] < n[c] for c in range(NCORES)):
            w = min(
                int(lists[c][1][ptr[c]]) // WIN
                for c in range(NCORES) if ptr[c] < n[c]
            )
            w = min(w, (BH - 128) // WIN)
            hi = WIN * w + 128
            for c in range(NCORES):
                li, dl = lists[c]
                j0 = ptr[c]
                j1 = min(int(np.searchsorted(dl, hi)), j0 + 128, n[c])
                take = j1 - j0
                idx_slot = np.zeros(128, np.int64)
                rel_slot = np.full(128, -1.0, np.float32)
                idx_slot[:take] = li[j0:j1]
                rel_slot[:take] = dl[j0:j1] - WIN * w
                core_idx[c].append(idx_slot)
                core_rel[c].append(rel_slot)
                ptr[c] = j1
            wins.append(w)
        sched[(h, ch, t)] = wins

    total_tiles = sum(len(sched[g]) for g in group_order)

    eidx = []
    rel = []
    for c in range(NCORES):
        slots = np.concatenate(core_idx[c]) if core_idx[c] else np.zeros(0, np.int64)
        eidx.append(_wrap16(slots))
        rel.append(np.stack(core_rel[c], axis=1).astype(np.float16))  # [128, ntiles]

    # output-node (emb) gather per half: stage 1 chunk-sorted into a DRAM
    # scratch, stage 2 in-order
    on_ch = {}
    for c in range(NCORES):
        for h in range(H):
            on_ch[(c, h)] = output_nodes[BC * c + BH * h:
                                         BC * c + BH * (h + 1)] // CHUNK
    M = {}      # (h, ch) -> padded stage-1 count (shared across cores)
    for h in range(H):
        for ch in range(NCH):
            mx = max(int(np.sum(on_ch[(c, h)] == ch)) for c in range(NCORES))
            M[(h, ch)] = max(round_up_to_multiple(mx, 16), 16)
    kch = {k: int(-(-m // 128)) for k, m in M.items()}   # scratch blocks
    scr_base = {}
    acc = 0
    for h in range(H):
        for ch in range(NCH):
            scr_base[(h, ch)] = acc
            acc += 128 * kch[(h, ch)]
    scr_rows = acc
    assert scr_rows <= CHUNK

    gidx1 = []   # stage-1 idx arrays, concatenated over (h, ch)
    gmap = []    # stage-2 idx (scratch slot per local b), [BC] (h-major)
    for c in range(NCORES):
        parts = []
        slots_all = []
        for h in range(H):
            nodes = output_nodes[BC * c + BH * h:BC * c + BH * (h + 1)]
            chs = on_ch[(c, h)]
            order = np.argsort(chs, kind="stable")
            slots = np.zeros(BH, np.int64)
            for ch in range(NCH):
                sel = order[np.searchsorted(chs[order], ch):
                            np.searchsorted(chs[order], ch + 1)]
                col = np.zeros(M[(h, ch)], np.int64)
                col[:len(sel)] = nodes[sel] % CHUNK
                parts.append(col)
                slots[sel] = scr_base[(h, ch)] + np.arange(len(sel))
            slots_all.append(slots)
        gidx1.append(_wrap16(np.concatenate(parts)))
        gmap.append(_wrap16(np.concatenate(slots_all)))

    shared = dict(
        group_order=group_order,
        sched=sched,
        total_tiles=total_tiles,
        M=M,
        kch=kch,
        scr_base=scr_base,
        scr_rows=scr_rows,
    )
    percore = [
        dict(eidx=eidx[c], rel=rel[c], gidx1=gidx1[c], gmap=gmap[c])
        for c in range(NCORES)
    ]
    return shared, percore


def _build(nc, S):
    """Build the bass program (identical for all cores; data differs)."""
    group_order = S["group_order"]
    sched = S["sched"]
    TOT_TILES = S["total_tiles"]
    TOT_SLOTS = TOT_TILES * 128
    M, KCH, SCR_BASE, SCR_ROWS = S["M"], S["kch"], S["scr_base"], S["scr_rows"]
    M1_TOT = sum(M.values())

    table = nc.dram_tensor("table", [NUM_NODES, T * U], F16, kind="ExternalInput")
    embt = nc.dram_tensor("embt", [NUM_NODES, D], F32, kind="ExternalInput")
    eidx_t = nc.dram_tensor("eidx", [128, TOT_SLOTS // 16], I16, kind="ExternalInput")
    rel_t = nc.dram_tensor("rel", [128, TOT_TILES], F16, kind="ExternalInput")
    nrel_t = nc.dram_tensor("nrel", [128, TOT_TILES], F16, kind="ExternalInput")
    gidx1_t = nc.dram_tensor("gidx1", [128, M1_TOT // 16], I16, kind="ExternalInput")
    gmap_t = nc.dram_tensor("gmap", [128, BC // 16], I16, kind="ExternalInput")
    iota_t = nc.dram_tensor("iota", [128, 128], F16, kind="ExternalInput")
    w1s_t = nc.dram_tensor("w1s", [128, A], F32, kind="ExternalInput")
    w2s_t = nc.dram_tensor("w2s", [128, 1], F32, kind="ExternalInput")
    ind4_t = nc.dram_tensor("ind4", [128, T], F32, kind="ExternalInput")
    ind4t_t = nc.dram_tensor("ind4t", [T, 128], F32, kind="ExternalInput")
    ones4_t = nc.dram_tensor("ones4", [T, 1], F32, kind="ExternalInput")
    ones1_t = nc.dram_tensor("ones1", [1, 128], F32, kind="ExternalInput")
    isum_t = nc.dram_tensor("isum", [128, U], F32, kind="ExternalInput")
    wsb_t = nc.dram_tensor("wsb", [U, T * D], F32, kind="ExternalInput")
    out_t = nc.dram_tensor("out", [BC, T, D], F32, kind="ExternalOutput")
    scratch = nc.dram_tensor("embscr", [SCR_ROWS, D], F32, kind="Internal")

    # per-group eidx column starts (in idx slots / 16)
    g_ntk = [len(sched[g]) for g in group_order]
    g_col = np.concatenate([[0], np.cumsum([n * 128 for n in g_ntk])])
    g_tcol = np.concatenate([[0], np.cumsum(g_ntk)])
    NGRP_H = len(group_order) // H

    with tile.TileContext(nc) as tc:
      with contextlib.ExitStack() as ctx2:
        cpool = ctx2.enter_context(tc.tile_pool(name="const", bufs=1))
        # constants first: the onehot path (iota/rel) must not queue behind
        # the bulk eidx upload on the in-order SP DMA queue
        eidx_sb = cpool.tile([128, TOT_SLOTS // 16], I16)
        c1_first = int(g_col[min(4, len(group_order))] // 16)
        if c1_first > 0:
            nc.sync.dma_start(eidx_sb[:, 0:c1_first], eidx_t[:, 0:c1_first])
        iota_sb = cpool.tile([128, 128], F16)
        nc.sync.dma_start(iota_sb[:], iota_t[:])
        rel_sb = cpool.tile([128, TOT_TILES], F16)
        nrel_sb = cpool.tile([128, TOT_TILES], F16)
        hcut = int(g_tcol[NGRP_H])
        nc.sync.dma_start(rel_sb[:, 0:hcut], rel_t[:, 0:hcut])
        nc.sync.dma_start(nrel_sb[:, 0:hcut], nrel_t[:, 0:hcut])
        w1s_sb = cpool.tile([128, A], F32)
        nc.sync.dma_start(w1s_sb[:], w1s_t[:])
        w2s_sb = cpool.tile([128, 1], F32)
        nc.sync.dma_start(w2s_sb[:], w2s_t[:])
        ind4_sb = cpool.tile([128, T], F32)
        nc.sync.dma_start(ind4_sb[:], ind4_t[:])
        ind4t_sb = cpool.tile([T, 128], F32)
        nc.sync.dma_start(ind4t_sb[:], ind4t_t[:])
        ones4_sb = cpool.tile([T, 1], F32)
        nc.sync.dma_start(ones4_sb[:], ones4_t[:])
        ones1_sb = cpool.tile([1, 128], F32)
        nc.sync.dma_start(ones1_sb[:], ones1_t[:])
        isum_sb = cpool.tile([128, U], F32)
        nc.sync.dma_start(isum_sb[:], isum_t[:])
        wsb_sb = cpool.tile([U, T * D], F32)
        nc.sync.dma_start(wsb_sb[:], wsb_t[:])
        gidx1_sb = cpool.tile([128, M1_TOT // 16], I16)
        nc.sync.dma_start(gidx1_sb[:], gidx1_t[:])
        gmap_sb = cpool.tile([128, BC // 16], I16)
        nc.sync.dma_start(gmap_sb[:], gmap_t[:])
        # bulk edge idx upload in ~16 coalesced slices (group-aligned)
        NG = len(group_order)
        gcuts = [min(4, NG)]
        while gcuts[-1] < NG:
            gcuts.append(min(gcuts[-1] + 8, NG))
        for a, b in zip(gcuts[:-1], gcuts[1:]):
            c0, c1 = int(g_col[a] // 16), int(g_col[b] // 16)
            if c1 > c0:
                nc.sync.dma_start(eidx_sb[:, c0:c1], eidx_t[:, c0:c1])
        nc.sync.dma_start(rel_sb[:, hcut:], rel_t[:, hcut:])
        nc.sync.dma_start(nrel_sb[:, hcut:], nrel_t[:, hcut:])
        aggT_sb = [cpool.tile([128, BH], F32, name=f"aggsb{h}") for h in range(H)]
        embB = [cpool.tile([128, BH // 128, D], F32, name=f"embB{h}")
                for h in range(H)]

        # ---- emb gather stage 1 (interleaved into the edge stream) -------
        scr_writes = [[] for _ in range(H)]
        g1pool = ctx2.enter_context(tc.tile_pool(name="g1", bufs=3))
        _off1 = [0]

        def _emb_stage1_call(h, ch):
            m, k = M[(h, ch)], KCH[(h, ch)]
            off1 = _off1[0]
            ge = g1pool.tile([128, k, D], F32, name=f"ge{h}_{ch}", tag="ge")
            in_view = embt[ch * CHUNK:min(NUM_NODES, (ch + 1) * CHUNK), :]
            _dma_gather_raw(nc.gpsimd, ge[:], in_view,
                            gidx1_sb[:, off1 // 16:(off1 + m) // 16],
                            m, D, D, queue_num=ch % 4)
            wr = nc.sync.dma_start(
                scratch[SCR_BASE[(h, ch)]:SCR_BASE[(h, ch)] + 128 * k, :]
                .rearrange("(k p) d -> p k d", p=128),
                ge[:],
            )
            scr_writes[h].append(wr)
            _off1[0] = off1 + m

        # pools for the edge stream + attention/output (kept open throughout)
        mpool = ctx2.enter_context(tc.tile_pool(name="msgs", bufs=10))
        ohpool = ctx2.enter_context(tc.tile_pool(name="ohp", bufs=6))
        aggps = ctx2.enter_context(tc.psum_pool(name="aggps", bufs=2))
        php = ctx2.enter_context(tc.psum_pool(name="ph", bufs=2))
        wkp = ctx2.enter_context(tc.tile_pool(name="wk", bufs=2))
        pop = ctx2.enter_context(tc.psum_pool(name="po", bufs=2))
        owp = ctx2.enter_context(tc.tile_pool(name="ow", bufs=2))

        aggT_ps = [aggps.tile([128, BH], F32, name=f"aggps{h}", tag=f"agg{h}")
                   for h in range(H)]
        for h in range(H):
            nc.vector.memset(aggT_ps[h][:], 0.0)
        # pre-clear msgs slots once (stale tails must stay finite)
        max_ntk = max(g_ntk) if g_ntk else 1
        for i in range(10):
            mz = mpool.tile([128, max_ntk, U], F16, name=f"mz{i}", tag="msgs")
            nc.vector.memset(mz.rearrange("p a b -> p (a b)"), 0.0)

        def _attention_output(h):
            """Attention over types + transform + emb add + normalize +
            store for half h (aggT_ps[h] complete at this point)."""
            nc.vector.tensor_copy(aggT_sb[h][:], aggT_ps[h][:])
            psumH = php.tile([128, BH], F32, tag="ph")
            for t in range(T):
                nc.tensor.matmul(
                    psumH[32 * t:32 * (t + 1), :],
                    w1s_sb[32 * t:32 * (t + 1), :],
                    aggT_sb[h][32 * t:32 * (t + 1), :],
                    start=True, stop=True, tile_position=(32 * t, 32 * t))
            H_sb = wkp.tile([128, BH], F32, tag="wk")
            nc.scalar.activation(H_sb[:], psumH[:],
                                 mybir.ActivationFunctionType.Tanh)
            HW2 = wkp.tile([128, BH], F32, tag="wk")
            nc.vector.tensor_scalar(HW2[:], H_sb[:], w2s_sb[:, 0:1], None,
                                    mybir.AluOpType.mult)
            psumS = php.tile([128, BH], F32, tag="ph")
            nc.tensor.matmul(psumS[0:T, :], ind4_sb[:], HW2[:],
                             start=True, stop=True)
            E4 = wkp.tile([T, BH], F32, tag=f"e4_{h}", bufs=1)
            nc.scalar.activation(E4[:], psumS[0:T, :],
                                 mybir.ActivationFunctionType.Exp)
            psumD = php.tile([128, BH], F32, tag="ph")
            nc.tensor.matmul(psumD[0:1, :], ones4_sb[:], E4[:],
                             start=True, stop=True)
            r_sb = wkp.tile([1, BH], F32, tag=f"r_{h}", bufs=1)
            nc.vector.reciprocal(r_sb[:], psumD[0:1, :])
            psumE = php.tile([128, BH], F32, tag="ph")
            nc.tensor.matmul(psumE[:], ind4t_sb[:], E4[:],
                             start=True, stop=True)
            psumR = php.tile([128, BH], F32, tag="ph")
            nc.tensor.matmul(psumR[:], ones1_sb[:], r_sb[:],
                             start=True, stop=True)
            M_sb = wkp.tile([128, BH], F32, tag="wk")
            nc.vector.tensor_tensor(M_sb[:], aggT_sb[h][:], psumE[:],
                                    mybir.AluOpType.mult)
            nc.vector.tensor_tensor(M_sb[:], M_sb[:], psumR[:],
                                    mybir.AluOpType.mult)
            psumC = php.tile([128, BH], F32, tag="ph")
            nc.tensor.matmul(psumC[0:U, :], isum_sb[:], M_sb[:],
                             start=True, stop=True)
            combT = wkp.tile([U, BH], F32, tag=f"comb_{h}", bufs=1)
            nc.vector.tensor_copy(combT[:], psumC[0:U, :])

            for j in range(BH // 128):
                psumO = pop.tile([128, T * D], F32, tag="po")
                for t in range(T):
                    nc.tensor.matmul(
                        psumO[:, D * t:D * (t + 1)],
                        combT[:, 128 * j:128 * (j + 1)],
                        wsb_sb[:, D * t:D * (t + 1)],
                        start=True, stop=True)
                O_sb = owp.tile([128, T * D], F32, tag="osb")
                eslice = embB[h][:, j, :]
                emb_bc = _bc_ap(eslice,
                                [eslice.ap[0], [0, T], eslice.ap[1]])
                nc.vector.tensor_tensor(
                    O_sb.rearrange("p (t d) -> p t d", t=T),
                    psumO.rearrange("p (t d) -> p t d", t=T), emb_bc,
                    mybir.AluOpType.add)
                sq = owp.tile([128, T * D], F32, tag="sq")
                nc.scalar.activation(sq[:], O_sb[:],
                                     mybir.ActivationFunctionType.Square)
                ss = owp.tile([128, T], F32, tag="ss")
                nc.vector.tensor_reduce(
                    ss[:], sq.rearrange("p (t d) -> p t d", t=T),
                    mybir.AxisListType.X, mybir.AluOpType.add)
                nrm = owp.tile([128, T], F32, tag="nrm")
                nc.scalar.activation(nrm[:], ss[:],
                                     mybir.ActivationFunctionType.Sqrt)
                nc.vector.tensor_scalar(nrm[:], nrm[:], 1e-12, None,
                                        mybir.AluOpType.max)
                rn = owp.tile([128, T], F32, tag="rn")
                nc.vector.reciprocal(rn[:], nrm[:])
                rn_bc = _bc_ap(rn, [rn.ap[0], rn.ap[1], [0, D]])
                nc.vector.tensor_tensor(
                    O_sb.rearrange("p (t d) -> p t d", t=T),
                    O_sb.rearrange("p (t d) -> p t d", t=T), rn_bc,
                    mybir.AluOpType.mult)
                nc.sync.dma_start(
                    out_t.rearrange("(j p) t d -> p j (t d)", p=128)
                    [:, (BH // 128) * h + j, :],
                    O_sb[:])

        # ---- edge-message gather + matmul scatter, h-major ---------------
        ohcnt = 0
        for gi, (h, ch, t) in enumerate(group_order):
            ntk = g_ntk[gi]
            if ntk > 0:
                col = int(g_col[gi])
                tcol = int(g_tcol[gi])
                msgs = mpool.tile([128, max_ntk, U], F16, name=f"m{gi}",
                                  tag="msgs")
                in_view = table[ch * CHUNK:min(NUM_NODES, (ch + 1) * CHUNK),
                                U * t:U * (t + 1)]
                _dma_gather_raw(
                    nc.gpsimd,
                    msgs[:, :ntk, :],
                    in_view,
                    eidx_sb[:, col // 16:(col + ntk * 128) // 16],
                    ntk * 128, U, T * U, queue_num=gi % 4)
                for b0 in range(0, ntk, OH_BATCH):
                    nb = min(OH_BATCH, ntk - b0)
                    oh = ohpool.tile([128, OH_BATCH, 128], F16,
                                     name=f"oh{gi}_{b0}", tag="oh")
                    iota_bc = _bc_ap(iota_sb,
                                     [iota_sb.ap[0], [0, nb], [1, 128]])
                    if ohcnt % 3 != 2:
                        rsl = rel_sb[:, tcol + b0:tcol + b0 + nb]
                        rel_bc = _bc_ap(
                            rsl, [rsl.ap[0], [rsl.ap[1][0], nb], [0, 128]])
                        nc.vector.tensor_tensor(
                            oh[:, :nb, :], iota_bc, rel_bc,
                            mybir.AluOpType.is_equal)
                    else:
                        # onehot = Relu(1 - |iota - rel|) on the Scalar engine
                        for k2 in range(nb):
                            nc.scalar.activation(
                                oh[:, k2, :], iota_sb[:],
                                mybir.ActivationFunctionType.Abs,
                                bias=nrel_sb[:, tcol + b0 + k2:
                                             tcol + b0 + k2 + 1])
                        nc.scalar.activation(
                            oh[:, :nb, :], oh[:, :nb, :],
                            mybir.ActivationFunctionType.Relu,
                            bias=1.0, scale=-1.0)
                    ohcnt += 1
                    for k in range(nb):
                        w = sched[(h, ch, t)][b0 + k]
                        assert WIN * w + 128 <= BH
                        nc.tensor.matmul(
                            aggT_ps[h][32 * t:32 * (t + 1),
                                       WIN * w:WIN * w + 128],
                            msgs[:, b0 + k, :], oh[:, k, :],
                            start=False, stop=False, skip_group_check=True,
                            tile_position=(0, 32 * t))
            # interleave emb stage-1 gathers: after the last type of chunk ch
            if t == T - 1:
                _emb_stage1_call(h, ch)
            # end of half h: emb stage 2, then attention + output for h
            if (gi + 1) % NGRP_H == 0:
                g2 = _dma_gather_raw(
                    nc.gpsimd, embB[h][:], scratch[:, :],
                    gmap_sb[:, (BH // 16) * h:(BH // 16) * (h + 1)],
                    BH, D, D, queue_num=0)
                for wr in scr_writes[h]:
                    bass._add_dep_helper(
                        g2.ins, wr.ins, sync=True,
                        reason="emb stage2 waits on scratch writes")
                _attention_output(h)
    nc.finalize()
    return nc


def kernel(**inputs):
    shared, percore = _prep(inputs)

    table_f16 = np.ascontiguousarray(
        np.asarray(inputs["node_type_embeddings"], dtype=np.float32)
        .reshape(NUM_NODES, T * U)).astype(np.float16)
    embt = np.ascontiguousarray(
        np.asarray(inputs["node_embeddings"], dtype=np.float32))
    trans_w = np.asarray(inputs["trans_w"], dtype=np.float32)
    trans_w_s1 = np.asarray(inputs["trans_w_s1"], dtype=np.float32)
    trans_w_s2 = np.asarray(inputs["trans_w_s2"], dtype=np.float32)

    w1s = np.ascontiguousarray(trans_w_s1.reshape(T * U, A))
    w2s = np.ascontiguousarray(trans_w_s2.reshape(T * A, 1))
    ind4 = np.zeros((128, T), np.float32)
    for t in range(T):
        ind4[32 * t:32 * (t + 1), t] = 1.0
    ind4t = np.ascontiguousarray(ind4.T)
    ones4 = np.ones((T, 1), np.float32)
    ones1 = np.ones((1, 128), np.float32)
    isum = np.zeros((128, U), np.float32)
    for t in range(T):
        isum[32 * t:32 * (t + 1), :] = np.eye(U, dtype=np.float32)
    wsb = np.ascontiguousarray(
        trans_w.transpose(1, 0, 2).reshape(U, T * D))
    iota = np.tile(np.arange(128, dtype=np.float16), (128, 1))

    nc = bacc.Bacc(trn_type="TRN2", num_swdge_queues=4,
                   dynamic_dma_scratch_size=32768)
    _build(nc, shared)

    common = dict(table=table_f16, embt=embt, iota=iota, w1s=w1s, w2s=w2s,
                  ind4=ind4, ind4t=ind4t, ones4=ones4, ones1=ones1,
                  isum=isum, wsb=wsb)
    in_maps = []
    for c in range(NCORES):
        pc = percore[c]
        in_maps.append(dict(common, eidx=pc["eidx"], rel=pc["rel"],
                            nrel=-pc["rel"],
                            gidx1=pc["gidx1"], gmap=pc["gmap"]))

    res = run_bass_kernel_spmd(nc, in_maps, core_ids=list(range(NCORES)))
    out = np.concatenate([r["out"] for r in res.results], axis=0)
    return out.astype(np.float32)


if __name__ == "__main__":
    import reference
    inputs = reference.setup_inputs()
    inputs = {k: np.asarray(v) for k, v in inputs.items()}
    out = kernel(**inputs)
    print("out", out.shape, out.dtype)
